# revision 42
# baseline (speedup 1.0000x reference)
"""Trainium2 Bass kernel for the Dial2vec contrastive loss (nn_Dial2vec).

Math: the dense reference computes, per sequence,
    q = h * a[:,None]; r = h * b[:,None]               (a/b = role-0/1 masks)
    w = q @ r^T; fw = w * band                         (band from turn ids)
    q_cross = fw^T @ q; r_cross = fw @ r
then masked means of q / q_cross / r / r_cross, cosine similarities, and a
label-weighted log-softmax loss.

Because band[i,j] depends only on (turn_i, turn_j) and a*b == 0, everything
collapses to per-turn segment sums over the 16 turns:
    Q_T[t] = sum_{turn_l = t} a_l h_l;  R_T[t] likewise with b     [16, H]
    gam_l  = a_l (Band R_T)[turn_l].h_l + b_l (Band Q_T)[turn_l].h_l
    qs = sum a_l h_l; qc = sum a_l gam_l h_l; rs/rc likewise with b
and cosine similarity is scale-invariant, so the mask-count denominators
cancel and gam can carry an arbitrary power-of-two scale (1/16 here, to fit
fp8 range).

Device pipeline per 3-sequence group (data parallel over 8 cores, one
dialogue = 10 sequences per core, fp8 activations with fp32 PSUM accumulate):
  A  : QRT[32,H] = [A1|B1]^T @ h          (token contraction, PE col-tiled)
  T  : QRT^T via PE transpose-mode        (identity matmul; DMA-transpose is
                                           globally serialized vs other DMAs)
  Y' : Y[32,LC] = QRT @ h^T               (H contraction vs host-shipped
                                           H-major h, PE col-tiled)
  Z  : Z = abx . Y'/16                    (one fused DVE op, fp8)
  y  : per-token gam-mask cols = Z^T @ ones  (one tiny PE matmul per chunk)
  D  : [qs,rs,qc,rc] = [a,b,ya,yb]^T @ h  (token contraction, PE col-tiled)
The host performs index-only preprocessing (one-hot / band-smeared masks,
fp8 casts, both h layouts) and the final O(B*H) cosine/log-softmax
reduction over the 40 gathered fp32 vectors per core.
"""

import os

import numpy as np

B_SEQ = 80
L = 512
H = 768
SAMPLES = 10
T = 16
VIEW_RANGE = 2
TEMP = 0.2
AVG_EPS = 1e-6
COS_EPS = 1e-8

N_CORES = 8
SPC = SAMPLES  # sequences per core = one dialogue
P = 128
PCH = 96  # tokens per chunk
LC = 288  # compacted token count (attention_mask=1 tokens only, zero-padded)
CHUNKS = LC // PCH  # 3
HS = H // P  # 6 H-slices
N_SPLITS = ((0, 512), (512, 768))  # PSUM-bank-aligned fp32 free-dim splits
SC = 1.0 / 16  # gam scale (power of two; cancels in cosine)
WARM_N = 2  # PE warm-up matmuls (HAM ramp) before the first real stage

# 2T-row supergroups of 3 sequences (PSUM base partitions {0,32,64})
GROUPS = [list(range(g, min(g + 3, SPC))) for g in range(0, SPC, 3)]
NG = len(GROUPS)
HIW = CHUNKS * H  # per-seq token-major width (2304)
HTW = HS * LC  # per-seq H-major width (1728)
DCW = 3 * 2 * T + 4 * 3 * (CHUNKS - 1)  # dcg cols per group: 96 + 12 + 12
AB_W = SPC * CHUNKS * 2 * T  # 960
AX_O = AB_W + NG * DCW  # + 480
ON_O = AX_O + NG * LC  # + NG*288
AUXW = ON_O + 6

_CACHE: dict = {}


def _build_nc(repeat: int = 1):
    """Build + compile the per-core Bass program (identical on all cores)."""
    from contextlib import ExitStack

    import concourse.bacc as bacc
    import concourse.mybir as mybir
    import concourse.tile as tile

    f32 = mybir.dt.float32
    bf16 = mybir.dt.bfloat16
    f8 = mybir.dt.float8e4

    nc = bacc.Bacc(
        "TRN2",
        debug=False,
        enable_asserts=False,
        target_bir_lowering=False,
    )

    # partition-major layouts: a group load reads one contiguous
    # [rows, G*F] block per partition -> large DMA descriptors
    hid = nc.dram_tensor("hid", [PCH, SPC, HIW], f8, kind="ExternalInput").ap()
    hidT = nc.dram_tensor("hidT", [P, SPC, HTW], f8, kind="ExternalInput").ap()
    # every small operand rides in ONE fp8 tensor / ONE DMA:
    # [ab: SPC*96 | dcg: NG*DCW | abx: NG*LC | ones: 6]
    aux = nc.dram_tensor("aux", [P, AUXW], f8, kind="ExternalInput").ap()
    idn = nc.dram_tensor("idn", [3 * 2 * T, 3 * 2 * T], bf16, kind="ExternalInput").ap()
    out = nc.dram_tensor("out", [4 * SPC, H], f32, kind="ExternalOutput").ap()

    with tile.TileContext(nc) as tc, ExitStack() as ctx:
        hip = ctx.enter_context(tc.tile_pool(name="hip", bufs=NG))
        htp = ctx.enter_context(tc.tile_pool(name="htp", bufs=NG))
        sap = ctx.enter_context(tc.tile_pool(name="sap", bufs=2))
        t8p = ctx.enter_context(tc.tile_pool(name="t8p", bufs=2))
        zp = ctx.enter_context(tc.tile_pool(name="zp", bufs=2))
        osp = ctx.enter_context(tc.tile_pool(name="osp", bufs=2))
        onp = ctx.enter_context(tc.tile_pool(name="onp", bufs=1))
        # PSUM budget (8 banks): pps 3x2 (pA/pD) + pyp 1x1 + ppt 1x1
        pps = ctx.enter_context(tc.tile_pool(name="pps", bufs=3, space="PSUM"))
        pyp = ctx.enter_context(tc.tile_pool(name="pyp", bufs=1, space="PSUM"))
        ppt = ctx.enter_context(tc.tile_pool(name="ppt", bufs=1, space="PSUM"))

        # aux + identity ride FIRST on the sync ring: tiny, gate stage A
        auxt = onp.tile([P, AUXW], f8, name="aux", tag="aux")
        nc.sync.dma_start(auxt[:], aux)
        idt = onp.tile([3 * 2 * T, 3 * 2 * T], bf16, name="idn", tag="id")
        nc.sync.dma_start(idt[:], idn)

        for rep in range(repeat):
            st: dict = {}

            def emit_loads(gi):
                grp = GROUPS[gi]
                G = len(grp)
                s0 = grp[0]
                hig = hip.tile([PCH, 3 * HIW], f8, name=f"hi{rep}_{gi}", tag="hi")
                if gi == 0:
                    # per-seq pieces: A(0, j=0) starts after one 221KB load
                    for j in range(G):
                        nc.sync.dma_start(
                            hig[:, HIW * j : HIW * (j + 1)],
                            hid[:, s0 + j, :],
                        )
                else:
                    nc.sync.dma_start(
                        hig[:, 0 : G * HIW],
                        hid[:, s0 : s0 + G, :].rearrange("p s f -> p (s f)"),
                    )
                htg = htp.tile([P, 3 * HTW], f8, name=f"ht{rep}_{gi}", tag="ht")
                nc.sync.dma_start(
                    htg[:, 0 : G * HTW],
                    hidT[:, s0 : s0 + G, :].rearrange("p s f -> p (s f)"),
                )
                st[gi] = {"hig": hig, "htg": htg, "G": G}

            def hidv(gi, j, c, n0=0, n1=H):
                hig = st[gi]["hig"]
                base = HIW * j + H * c
                return hig[:, base + n0 : base + n1]

            def hidTv(gi, j, k):
                htg = st[gi]["htg"]
                base = HTW * j + LC * k
                return htg[:, base : base + LC]

            def abw(s, c):
                # stage-A weights of sequence s, chunk c
                o = 96 * s + 32 * c
                return auxt[0:PCH, o : o + 32]

            def dcol(gi, j, c):
                # stage-D weight cols [a, b, ya, yb] of group gi seq j chunk c
                o = AB_W + DCW * gi
                if c == 0:
                    return auxt[0:PCH, o + 32 * j : o + 32 * j + 32]
                o += 3 * 2 * T + 12 * (c - 1)
                return auxt[0:PCH, o + 4 * j : o + 4 * j + 4]

            def emit_A(gi, warm=False):
                # stage A: QRT = [A1|B1]^T @ h (PE, col-tiled over seqs)
                grp = GROUPS[gi]
                G = len(grp)
                GP = 32 * G
                pA = pps.tile([P, H], f32, name=f"pA{rep}_{gi}", tag="p")
                if warm:
                    # HAM warm-up: junk matmuls on the (early-loaded) aux
                    # tile keep the PE busy while the h stream lands
                    for _ in range(WARM_N):
                        nc.tensor.matmul(
                            pA[0:PCH, 0:512],
                            auxt[0:PCH, 0:PCH],
                            auxt[0:PCH, 0:512],
                            start=True,
                            stop=True,
                            skip_group_check=True,
                        )
                for j in range(G):
                    for c in range(CHUNKS):
                        lw = abw(grp[j], c)
                        for n0, n1 in N_SPLITS:
                            nc.tensor.matmul(
                                pA[32 * j : 32 * j + 32, n0:n1],
                                lw,
                                hidv(gi, j, c, n0, n1),
                                start=(c == 0),
                                stop=(c == CHUNKS - 1),
                                skip_group_check=warm,
                            )
                # QRT -> bf16 SBUF (DVE), consumed by the PE transposes
                sbA = sap.tile([3 * 2 * T, H], bf16, name=f"sA{rep}_{gi}", tag="sa")
                nc.vector.tensor_copy(sbA[0:GP, :], pA[0:GP, :])
                st[gi]["sbA"] = sbA

            def emit_T(gi):
                # QRT^T per H-slice via PE transpose-mode (identity matmul);
                # one DVE copy casts the bf16 psum result to fp8 SBUF
                grp = GROUPS[gi]
                G = len(grp)
                GP = 32 * G
                sbA = st[gi]["sbA"]
                pT = ppt.tile([P, HS * 3 * 2 * T], bf16, name=f"pT{rep}_{gi}", tag="pt")
                for c in range(HS):
                    nc.tensor.transpose(
                        pT[:, 96 * c : 96 * c + GP],
                        sbA[0:GP, 128 * c : 128 * c + 128],
                        idt[0:GP, 0:GP],
                    )
                t8 = t8p.tile([P, HS * 3 * 2 * T], f8, name=f"t8{rep}_{gi}", tag="t8")
                t8v = t8[:].rearrange("p (k c) -> p k c", k=HS)
                pTv = pT[:].rearrange("p (k c) -> p k c", k=HS)
                nc.vector.tensor_copy(t8v[:, :, 0:GP], pTv[:, :, 0:GP])
                st[gi]["t8"] = t8

            def emit_Y(gi):
                # Y' = QRT @ h^T (PE, col-tiled over seqs)
                grp = GROUPS[gi]
                G = len(grp)
                GP = 32 * G
                t8 = st[gi]["t8"]
                pY = pyp.tile([P, 512], f32, name=f"pY{rep}_{gi}", tag="py")
                for j in range(G):
                    for k in range(HS):
                        nc.tensor.matmul(
                            pY[32 * j : 32 * j + 32, 0:LC],
                            t8[:, 96 * k + 32 * j : 96 * k + 32 * j + 32],
                            hidTv(gi, j, k),
                            start=(k == 0),
                            stop=(k == HS - 1),
                        )
                st[gi]["pY"] = pY

            def emit_Zy(gi):
                # Z = abx . Y'/16 (one fused DVE op, fp8 out);
                # y cols = Z^T @ ones (PE); scatter into aux (ACT)
                grp = GROUPS[gi]
                G = len(grp)
                GP = 32 * G
                pY = st[gi]["pY"]
                zt = zp.tile([3 * 2 * T, LC], f8, name=f"z{rep}_{gi}", tag="z")
                # per-chunk Z pieces let each y-matmul start as soon as its
                # slice of the DVE multiply lands
                for c in range(CHUNKS):
                    nc.vector.scalar_tensor_tensor(
                        zt[0:GP, PCH * c : PCH * c + PCH],
                        pY[0:GP, PCH * c : PCH * c + PCH],
                        SC,
                        auxt[0:GP, AX_O + LC * gi + PCH * c : AX_O + LC * gi + PCH * (c + 1)],
                        mybir.AluOpType.mult,
                        mybir.AluOpType.mult,
                    )
                    nc.tensor.matmul(
                        pY[0:PCH, 488 + 6 * c : 488 + 6 * c + 6],
                        zt[0:GP, PCH * c : PCH * c + PCH],
                        auxt[0:GP, ON_O : ON_O + 6],
                        start=True,
                        stop=True,
                        skip_group_check=True,
                    )
                for c in range(CHUNKS):
                    pyv = pY[0:PCH, 488 + 6 * c : 488 + 6 * c + 6].rearrange(
                        "p (s q) -> p s q", q=2
                    )
                    o = AB_W + DCW * gi
                    if c == 0:
                        dv = auxt[0:PCH, o : o + 32 * G].rearrange(
                            "p (s q) -> p s q", q=32
                        )
                    else:
                        o += 3 * 2 * T + 12 * (c - 1)
                        dv = auxt[0:PCH, o : o + 4 * G].rearrange(
                            "p (s q) -> p s q", q=4
                        )
                    nc.scalar.copy(dv[:, :, 2:4], pyv[:, 0:G, :])

            def emit_D(gi):
                # stage D: [qs,rs,qc,rc] = [a,b,ya,yb]^T @ h (PE, col-tiled)
                grp = GROUPS[gi]
                G = len(grp)
                GP = 32 * G
                pD = pps.tile([P, H], f32, name=f"pD{rep}_{gi}", tag="p")
                for j in range(G):
                    for c in range(CHUNKS):
                        rows = 2 * T if c == 0 else 4
                        lw = dcol(gi, j, c)
                        for n0, n1 in N_SPLITS:
                            nc.tensor.matmul(
                                pD[32 * j : 32 * j + rows, n0:n1],
                                lw,
                                hidv(gi, j, c, n0, n1),
                                start=(c == 0),
                                stop=(c == CHUNKS - 1),
                                skip_group_check=True,
                            )
                osb = osp.tile([3 * 2 * T, H], f32, name=f"o{rep}_{gi}", tag="o")
                nc.scalar.copy(osb[0:GP, :], pD[0:GP, :])
                # outputs ride the scalar HWDGE queue (naturally ordered
                # right after the copy; the sync ring stays load-only)
                for j, s in enumerate(grp):
                    nc.scalar.dma_start(
                        out[4 * s : 4 * s + 4, :], osb[32 * j : 32 * j + 4, :]
                    )

            # all loads issue up front (sync ring drains in need order); the
            # software pipeline keeps the PE fed across the copyA/DVE
            # round-trips between each group's stages
            for gi in range(NG):
                emit_loads(gi)
            emit_A(0, warm=True)
            emit_A(1)
            emit_T(0)
            for gi in range(NG):
                emit_Y(gi)
                emit_Zy(gi)
                if gi + 2 < NG:
                    emit_A(gi + 2)
                if gi + 1 < NG:
                    emit_T(gi + 1)
                emit_D(gi)

    nc.compile()
    return nc


def _prep_core_inputs(hidden_states, attention_mask, role_ids, turn_ids):
    """Per-core input maps: one-hot / band-smeared mask prep (index work only)."""
    import ml_dtypes

    bf16 = ml_dtypes.bfloat16
    f8 = ml_dtypes.float8_e4m3

    active = attention_mask != 0
    counts = active.sum(-1)
    assert counts.max() <= LC, f"active tokens {counts.max()} exceed LC={LC}"
    # stable-sort active tokens to the front, keep the first LC positions.
    # Padded positions carry real h values but zero masks, so every
    # contribution they could make is exactly zero.
    sel = np.argsort(~active, axis=1, kind="stable")[:, :LC]  # [B, LC]

    am = np.take_along_axis(active, sel, axis=1).astype(np.float32)
    ro = np.take_along_axis(role_ids, sel, axis=1)
    tu = np.take_along_axis(turn_ids, sel, axis=1)
    hidden_states = np.take_along_axis(hidden_states, sel[..., None], axis=1)

    a = am * (ro == 0)
    b = am * (ro == 1)
    onehot = (tu[..., None] == np.arange(T, dtype=tu.dtype)).astype(
        np.float32
    )  # [B, LC, T]
    A1 = onehot * a[..., None]
    B1 = onehot * b[..., None]
    band = (
        np.abs(np.arange(T)[:, None] - np.arange(T)[None, :]) <= VIEW_RANGE
    ).astype(np.float32)
    A1b = A1 @ band  # a_l * band[turn_l, :]
    B1b = B1 @ band

    h8 = hidden_states.astype(f8)  # [B, LC, H]
    # token-major: [B, PCH, CHUNKS*H]
    hid = (
        h8.reshape(B_SEQ, CHUNKS, PCH, H).transpose(0, 2, 1, 3).reshape(B_SEQ, PCH, -1)
    )
    # H-major: [B, P, HS*LC]
    hidT = (
        h8.transpose(0, 2, 1)
        .reshape(B_SEQ, HS, P, LC)
        .transpose(0, 2, 1, 3)
        .reshape(B_SEQ, P, -1)
    )

    # stage-A weights: per chunk the [A1(16) | B1(16)] columns
    ab_full = (
        np.concatenate([A1, B1], axis=-1)
        .reshape(B_SEQ, CHUNKS, PCH, 2 * T)
        .transpose(0, 2, 1, 3)
        .reshape(B_SEQ, PCH, -1)
    )

    abx = np.ascontiguousarray(
        np.concatenate([B1b, A1b], axis=-1).transpose(0, 2, 1)
    )  # [B, 2T, LC]

    # stage-D weight template per group: [a, b, 0, 0] cols; chunk 0 padded
    # to 32 cols so stage D's start=True initializes the full psum strip
    ab2 = np.stack([a, b], axis=-1).reshape(B_SEQ, CHUNKS, PCH, 2)
    dcg_all = np.zeros((N_CORES, NG, PCH, DCW), np.float32)
    for gi, grp in enumerate(GROUPS):
        for j, sj in enumerate(grp):
            for core in range(N_CORES):
                s = core * SPC + sj
                dcg_all[core, gi, :, 32 * j : 32 * j + 2] = ab2[s, 0]
                for c in range(1, CHUNKS):
                    o = 3 * 2 * T + 12 * (c - 1) + 4 * j
                    dcg_all[core, gi, :, o : o + 2] = ab2[s, c]

    # ones pattern (scale lives in the Z op): for seq strip j, rows
    # 32j..32j+16 are the b*gam half (abx rows B1b x Y'_Q), rows
    # 32j+16..32j+32 the a*gam half.
    # dct col order is [a, b, a*gam, b*gam] -> py col 2j = a*gam, 2j+1 = b*gam
    ones3 = np.zeros((P, 6), np.float32)
    for j in range(3):
        ones3[32 * j + T : 32 * j + 2 * T, 2 * j] = 1.0  # a*gam
        ones3[32 * j : 32 * j + T, 2 * j + 1] = 1.0  # b*gam
    idn = np.eye(3 * 2 * T, dtype=bf16)

    in_maps = []
    for c in range(N_CORES):
        sl = slice(c * SPC, (c + 1) * SPC)
        auxm = np.zeros((P, AUXW), np.float32)
        auxm[0:PCH, 0:AB_W] = (
            ab_full[sl].transpose(1, 0, 2).reshape(PCH, AB_W)
        )
        auxm[0:PCH, AB_W:AX_O] = dcg_all[c].transpose(1, 0, 2).reshape(PCH, NG * DCW)
        for gi, grp in enumerate(GROUPS):
            for j, sj in enumerate(grp):
                auxm[32 * j : 32 * j + 32, AX_O + LC * gi : AX_O + LC * (gi + 1)] = (
                    abx[c * SPC + sj]
                )
        auxm[:, ON_O:] = ones3
        in_maps.append(
            {
                "hid": np.ascontiguousarray(hid[sl].transpose(1, 0, 2)),
                "hidT": np.ascontiguousarray(hidT[sl].transpose(1, 0, 2)),
                "aux": auxm.astype(f8),
                "idn": idn,
            }
        )
    # cheap reference for a device-integrity check: qs/rs rows only,
    # recomputed on host in fp32 from the same fp8 values
    hf = h8.astype(np.float32)
    qs_ref = np.einsum("bl,blh->bh", a, hf)
    rs_ref = np.einsum("bl,blh->bh", b, hf)
    return in_maps, a.sum(-1), b.sum(-1), qs_ref, rs_ref


def _outputs_ok(outs, qs_ref, rs_ref):
    """Detect corrupted device runs: finite outputs + stage-D qs/rs match host."""
    vecs = np.concatenate(outs, axis=0).reshape(-1, 4, H)
    if not np.isfinite(vecs).all():
        return False
    for got, ref in ((vecs[:, 0], qs_ref), (vecs[:, 1], rs_ref)):
        num = np.linalg.norm(got - ref, axis=-1)
        den = np.linalg.norm(ref, axis=-1) + 1e-6
        if (num / den).max() > 0.05:
            return False
    return True


def _finalize(outs, labels, na, nb):
    """Host-side O(B*H) reduction: cosine, log-softmax, label-weighted loss."""
    vecs = np.concatenate(outs, axis=0).astype(np.float64).reshape(-1, 4, H)
    qs = vecs[:, 0] / (na + AVG_EPS)[:, None]
    rs = vecs[:, 1] / (nb + AVG_EPS)[:, None]
    qc = vecs[:, 2] / (nb + AVG_EPS)[:, None]  # carries the SC scale: cancels
    rc = vecs[:, 3] / (na + AVG_EPS)[:, None]

    def cos(x, y):
        nx = np.maximum(np.linalg.norm(x, axis=-1), COS_EPS)
        ny = np.maximum(np.linalg.norm(y, axis=-1), COS_EPS)
        return (x * y).sum(-1) / (nx * ny)

    logit_q = (cos(qs, qc) / TEMP).reshape(-1, SAMPLES)
    logit_r = (cos(rs, rc) / TEMP).reshape(-1, SAMPLES)

    def lsm(x):
        m = x.max(-1, keepdims=True)
        e = np.exp(x - m)
        return x - m - np.log(e.sum(-1, keepdims=True))

    lab = labels.astype(np.float64)
    loss_q = -np.mean(lsm(logit_q) * lab)
    loss_r = -np.mean(lsm(logit_r) * lab)
    return np.float32(loss_r + loss_q)


def kernel(hidden_states, labels, attention_mask, role_ids, turn_ids):
    import time

    from concourse.bass_utils import run_bass_kernel_spmd

    if "nc" not in _CACHE:
        _CACHE["nc"] = _build_nc()
    nc = _CACHE["nc"]

    in_maps, na, nb, qs_ref, rs_ref = _prep_core_inputs(
        np.asarray(hidden_states),
        np.asarray(attention_mask),
        np.asarray(role_ids),
        np.asarray(turn_ids),
    )
    trace = bool(os.environ.get("BASS_KERNEL_TRACE"))

    # the axon/NRT path very occasionally drops a run (device-unrecoverable
    # or corrupted output); validate cheaply and retry rather than fail
    outs = None
    for attempt in range(3):
        try:
            res = run_bass_kernel_spmd(
                nc, in_maps, core_ids=list(range(N_CORES)), trace=trace
            )
            cand = [res.results[c]["out"] for c in range(N_CORES)]
        except Exception:
            if attempt == 2:
                raise
            time.sleep(2.0)
            continue
        outs = cand
        if _outputs_ok(cand, qs_ref, rs_ref):
            break
    if trace:
        _CACHE["last_results"] = res
        print(
            f"[kernel] exec_time_ns={res.exec_time_ns} "
            f"mean_exec_time_ns={res.mean_exec_time_ns}"
        )
    return _finalize(outs, np.asarray(labels), na, nb)
